# revision 44
# baseline (speedup 1.0000x reference)
"""Trainium2 Bass kernel for the news-attention module (bf16 pipeline).

Computes, per batch b:
    hist = [history_repr | pos_emb[positions]]            [H, 500]
    cand = [candidate_repr | pos_emb[1]]                  [N, 500]
    hc = cand @ Wc.T ; hh = hist @ Wh.T                   [*, 200]
    a[n,h] = w2 . relu(hc[n] + hh[h] + b1)
    alpha = softmax_h(mask ? a : -1e9)
    out1 = alpha @ hist ; out2 = cand

Structure (v6):
  - all compute tensors bf16 (fp32 only in PSUM, softmax logits, outputs);
    gpsimd cast-DMAs load HBM fp32 -> SBUF bf16 directly.
  - GEMM contraction in 4 zero-padded chunks of K=128; inputs transposed
    on PE in [100,128] blocks.
  - position gather folded into matmuls: pos part of hh = onehot(pos) @ E
    with E = pos_emb @ Wh2.T; candidate pos part + b1 folded into a
    per-partition bias column c0 applied during PSUM evacuation (with the
    fp32->bf16 convert and element duplication for the DVE 2x fast path).
  - hidden built per 4-batch quad in [a-chunk, (b,n,h)] bf16 with
    zero-stride broadcast APs on the DVE; relu via tensor_scalar_max (4x).
  - w2 matvec in column form: each lhsT chunk is a contiguous 100-column
    (n-pair x 50 h) hidden slice (HW-measured ~32ns/chunk-instr vs 80 for
    128-col chunks); output lands dense on 100 PSUM partitions (parity,h).
  - mask bias folded into the matvec PSUM evacuation as a per-partition
    bias column; logits stay fp32 through transpose/softmax; softmax
    batched across batches in [25c, b, parity, h] layout.
  - final attention: per (batch, parity) PE transpose of bf16 probs ->
    [h, 25], matmul against bf16 hist rows with 1/sum folded into the
    PSUM evacuation (split ACT/DVE).
  - candidate passthrough entirely DRAM->DRAM (no SBUF round trip).
  - pools and SBUF tiles are allocated once and shared by all in-NEFF
    reps (no per-rep pool barriers), so back-to-back invocations pipeline.

Sharding: data-parallel over batch, 8 batches per core on 8 cores.
Params replicated. Full inputs in, full outputs out.
"""

import sys

for _p in ("/opt/trn_rl_repo",):
    if _p not in sys.path:
        sys.path.insert(0, _p)

import numpy as np

import concourse.bass as bass
import concourse.bacc as bacc
import concourse.tile as tile
from concourse import mybir
from concourse import bass_utils
from concourse.masks import make_identity

DT = mybir.dt.float32
BF = mybir.dt.bfloat16
I32 = mybir.dt.int32
AF = mybir.ActivationFunctionType
ALU = mybir.AluOpType
AX = mybir.AxisListType

NCORES = 8
B = 64
BC = B // NCORES  # 8 batches per core
H = 50
N = 50
D = 400
P = 100
A = 200
F = D + P       # 500
J = 52
NC2 = N // 2    # 25 n-pair chunks per batch
KC = 128        # GEMM contraction chunk rows

QB = 4  # batches per hidden-add instruction
SKIP = set()   # timing ablations: {"hidden","matvec","gemm","transp","final"}


def _bc(v, pos, n):
    """Insert a zero-stride (broadcast) dim of length n at position pos."""
    ap = [list(x) for x in v.ap]
    ap.insert(pos, [0, n])
    return bass.AP(tensor=v.tensor, offset=v.offset, ap=ap)


def _ap(v, offset_delta, ap_list):
    return bass.AP(tensor=v.tensor, offset=v.offset + offset_delta, ap=ap_list)


class _St:
    pass


def _setup(nc, tc, ctx):
    """Pools, SBUF tiles, and static constants -- created exactly once."""
    st = _St()
    st.consts = ctx.enter_context(tc.tile_pool(name="consts", bufs=1))
    st.ps = ctx.enter_context(tc.tile_pool(name="ps", bufs=1, space="PSUM"))
    st.psb = ctx.enter_context(tc.tile_pool(name="psb", bufs=1, space="PSUM"))
    st.psm = ctx.enter_context(tc.tile_pool(name="psm", bufs=2, space="PSUM"))
    st.pst = ctx.enter_context(tc.tile_pool(name="pst", bufs=1, space="PSUM"))
    st.pse = ctx.enter_context(tc.tile_pool(name="pse", bufs=1, space="PSUM"))
    st.purp = ctx.enter_context(tc.tile_pool(name="purp", bufs=2, space="PSUM"))
    st.amcp = ctx.enter_context(tc.tile_pool(name="amcp", bufs=2))
    st.eTp = ctx.enter_context(tc.tile_pool(name="eTp", bufs=4))
    consts = st.consts

    ident = consts.tile([128, 128], DT, name="ident")
    make_identity(nc, ident)
    identB = consts.tile([128, 128], BF, name="identB")
    make_identity(nc, identB)
    iot = consts.tile([J, BC * H], I32, name="iot")
    nc.gpsimd.iota(iot, pattern=[[0, BC * H]], base=0, channel_multiplier=1)
    one11 = consts.tile([1, 1], DT, name="one11")
    nc.vector.memset(one11, 1.0)
    st.ident, st.identB, st.iot, st.one11 = ident, identB, iot, one11

    st.cand_all = consts.tile([100, 4, 4 * KC], BF, name="cand_all")
    st.hist_all = consts.tile([100, 4, 4 * KC], BF, name="hist_all")
    # pad feature columns only need to be finite (they hit zero weights)
    nc.scalar.memzero(st.cand_all[:, :, D:4 * KC])
    nc.scalar.memzero(st.hist_all[:, :, D:4 * KC])
    st.pos52 = consts.tile([J, BC * H], I32, name="pos52")
    st.onehot_s = consts.tile([J, BC * H], BF, name="onehot_s")
    st.wpos = consts.tile([100, 2, A], BF, name="wpos")
    st.posT = consts.tile([P, J], BF, name="posT")
    st.pos_emb_s = consts.tile([J, P], BF, name="pos_emb_s")
    st.w1T = consts.tile([KC, 8, A], BF, name="w1T")
    nc.scalar.memzero(st.w1T)
    st.w2col = consts.tile([100, 2], BF, name="w2col")
    st.b1row = consts.tile([1, A], DT, name="b1row")
    st.mb2 = consts.tile([100, BC], DT, name="mb2")
    st.histf = consts.tile([H, BC, F], BF, name="histf")
    st.E_s = consts.tile([J, A], BF, name="E_s")
    st.c0col = consts.tile([100, 2], DT, name="c0col")
    st.candT = consts.tile([KC, 4, BC * N], BF, name="candT")
    st.histT = consts.tile([KC, 4, BC * H], BF, name="histT")
    st.hcT2 = consts.tile([100, 2, BC * N, 2], BF, name="hcT2")
    st.hhT = consts.tile([100, 2, BC * H], BF, name="hhT")
    st.nquad = BC // QB
    st.hids = [[consts.tile([100, QB, N, H], BF, name=f"hid{q}_{ac}")
                for ac in range(2)] for q in range(st.nquad)]
    st.amr = consts.tile([NC2, BC, 2, H], DT, name="amr")
    st.ex = consts.tile([NC2, BC, 2, 64], BF, name="ex")
    st.nm = consts.tile([NC2, BC, 2], DT, name="nm")
    st.am2 = consts.tile([NC2, BC, 2, H], DT, name="am2")
    st.rs = consts.tile([NC2, BC, 2], DT, name="rs")
    st.urs = consts.tile([NC2, 2, BC, F], DT, name="urs")
    return st


def _rep(nc, st, hist_in, cand_in, mask_in, pos_in, pos_emb, w1t, pos_embT,
         b1, w2, ur_out, cand_out):
    cand_all, hist_all, w1T = st.cand_all, st.hist_all, st.w1T
    candT, histT, hcT2, hhT = st.candT, st.histT, st.hcT2, st.hhT
    onehot_s, histf, mb2 = st.onehot_s, st.histf, st.mb2
    amr, ex, nm, am2, rs, urs = st.amr, st.ex, st.nm, st.am2, st.rs, st.urs
    ident, identB = st.ident, st.identB

    # ---------------- bf16 cast loads (gpsimd SWDGE) ----------------
    # queue order = need order: inputs, pos indices, weights, hist-final
    for hf in range(2):
        src_c = _ap(cand_in.ap(), hf * N * D,
                    [[D, 50], [2 * N * D, 4], [1, D]])
        nc.gpsimd.dma_start(out=cand_all[hf * 50:(hf + 1) * 50, :, 0:D],
                            in_=src_c)
    for hf in range(2):
        src_h = _ap(hist_in.ap(), hf * H * D,
                    [[D, 50], [2 * H * D, 4], [1, D]])
        nc.gpsimd.dma_start(out=hist_all[hf * 50:(hf + 1) * 50, :, 0:D],
                            in_=src_h)

    nc.gpsimd.dma_start(out=st.pos52, in_=_bc(pos_in.ap(), 0, J))
    nc.vector.tensor_tensor(out=onehot_s, in0=st.iot, in1=st.pos52,
                            op=ALU.is_equal)

    # Wc2 (f 400:500) and Wh2 (f 900:1000) as [100, 2, A]
    nc.gpsimd.dma_start(
        out=st.wpos, in_=_ap(w1t.ap(), D * A, [[A, 100], [F * A, 2], [1, A]]))
    nc.gpsimd.dma_start(out=st.posT, in_=pos_embT.ap())
    nc.gpsimd.dma_start(out=st.pos_emb_s, in_=pos_emb.ap())

    # W1T in 8 zero-padded K-chunks of 128: q0-3 cand f, q4-7 hist f
    for half in range(2):
        base = half * F
        nc.gpsimd.dma_start(
            out=w1T[:, 4 * half:4 * half + 3, :],
            in_=_ap(w1t.ap(), base * A, [[A, KC], [KC * A, 3], [1, A]]))
        nc.gpsimd.dma_start(
            out=w1T[0:F - 3 * KC, 4 * half + 3, :],
            in_=_ap(w1t.ap(), (base + 3 * KC) * A,
                    [[A, F - 3 * KC], [1, A]]))

    nc.gpsimd.dma_start(out=st.w2col,
                        in_=w2.ap().rearrange("(c p) -> p c", p=100))
    nc.sync.dma_start(out=st.b1row, in_=b1.ap())

    # mask bias (mask-1)*1e9 as per-batch bias columns in the matvec
    # output layout [(n-parity, h) partitions, b]
    for nr in range(2):
        nc.sync.dma_start(
            out=mb2[50 * nr:50 * nr + H, :],
            in_=_ap(mask_in.ap(), 0, [[1, H], [H, BC]]))
    nc.scalar.activation(out=mb2, in_=mb2, func=AF.Copy,
                         bias=-1e9, scale=1e9)

    # candidate passthrough entirely in DRAM (independent of everything)
    nc.sync.dma_start(
        out=_ap(cand_out.ap(), 0, [[N * F, BC], [F, N], [1, D]]),
        in_=cand_in.ap())

    # hist rows with position columns, bf16 (final-attention rhs; late)
    nc.gpsimd.dma_start(
        out=histf[:, :, 0:D],
        in_=_ap(hist_in.ap(), 0, [[D, H], [H * D, BC], [1, D]]))
    nc.gpsimd.dma_start(
        out=cand_out.ap()[:, :, D:F],
        in_=_bc(_bc(pos_emb.ap()[1:2, :], 0, N), 0, BC))

    # E[j, a] = pos_emb @ Wh2.T
    psE = st.ps.tile([J, A], DT, tag="ps")
    nc.tensor.matmul(psE, lhsT=st.posT[:, :], rhs=st.wpos[:, 1, :],
                     start=True, stop=True)
    nc.vector.tensor_copy(out=st.E_s, in_=psE)

    # c0[a] = Wc2 @ pos_emb[1] + b1 as two per-partition bias columns
    for ac in range(2):
        asl = slice(ac * 100, (ac + 1) * 100)
        psc = st.ps.tile([100, 1], DT, tag="ps")
        nc.tensor.matmul(psc, lhsT=st.wpos[:, 0, asl], rhs=st.posT[:, 1:2],
                         start=True, stop=False)
        nc.tensor.matmul(psc, lhsT=st.b1row[:, asl], rhs=st.one11[:, :],
                         start=False, stop=True)
        nc.scalar.copy(out=st.c0col[:, ac:ac + 1], in_=psc)

    # ---------------- input transposes (PE, 128-col chunks) ----------
    for k in range(4 if "transp" not in SKIP else 1):
        ptc = st.psb.tile([KC, 4, 100], BF, tag="psb")
        for g in range(4):
            nc.tensor.transpose(
                ptc[:, g, :],
                cand_all[:, g, k * KC:(k + 1) * KC],
                identB[:100, :100])
        nc.scalar.copy(out=candT[:, k, :], in_=ptc)
        pth = st.psb.tile([KC, 4, 100], BF, tag="psb")
        for g in range(4):
            nc.tensor.transpose(
                pth[:, g, :],
                hist_all[:, g, k * KC:(k + 1) * KC],
                identB[:100, :100])
        nc.scalar.copy(out=histT[:, k, :], in_=pth)

    # ------- GEMMs: hcT2[a, (b,n), dup2] (duplicated), hhT[a, (b,h)] --
    for ac in range(2 if "gemm" not in SKIP else 1):
        asl = slice(ac * 100, (ac + 1) * 100)
        pg = st.ps.tile([100, BC * N], DT, tag="ps")
        for k in range(4):
            nc.tensor.matmul(pg, lhsT=w1T[:, k, asl],
                             rhs=candT[:, k, :],
                             start=(k == 0), stop=(k == 3))
        # evacuate + c0 bias, duplicating each element (dup2 dim)
        nc.scalar.activation(out=hcT2[:, ac, :, :], in_=_bc(pg[:, :], 2, 2),
                             func=AF.Identity, bias=st.c0col[:, ac:ac + 1],
                             scale=1.0)

        ph = st.ps.tile([100, BC * H], DT, tag="ps")
        for k in range(4):
            nc.tensor.matmul(ph, lhsT=w1T[:, 4 + k, asl],
                             rhs=histT[:, k, :],
                             start=(k == 0), stop=False)
        nc.tensor.matmul(ph, lhsT=st.E_s[:, asl], rhs=onehot_s[:, :],
                         start=False, stop=True)
        nc.vector.tensor_copy(out=hhT[:, ac, :], in_=ph)

    # position part of histf via one-hot gather matmuls (4 per tile)
    for quad in range(BC // 4):
        ppg = st.ps.tile([H, 4, P], DT, tag="ps")
        for i in range(4):
            b = quad * 4 + i
            nc.tensor.matmul(ppg[:, i, :],
                             lhsT=onehot_s[:, b * H:(b + 1) * H],
                             rhs=st.pos_emb_s[:, :], start=True, stop=True)
        nc.scalar.copy(out=histf[:, 4 * quad:4 * quad + 4, D:F], in_=ppg)

    # ---------------- hidden + relu (bf16) ---------------------------
    for q in range(st.nquad):
        for ac in range(2):
            hid = st.hids[q][ac]
            pstp = list(hid.ap[0])
            out_v = _ap(hid, 0, [pstp, [H, QB * N], [2, H // 2], [1, 2]])
            v = hcT2[:, ac, q * QB * N:(q + 1) * QB * N, :]
            hcb = _bc(v, 2, H // 2)          # [p, QB*N, 25, 2]
            w = hhT[:, ac, q * QB * H:(q + 1) * QB * H]
            hhb = _ap(w, 0, [list(w.ap[0]), [H, QB], [0, N],
                             [2, H // 2], [1, 2]])
            if "hidden" not in SKIP:
                nc.vector.tensor_add(out=out_v, in0=hcb, in1=hhb)
                flat = _ap(hid, 0, [pstp, [1, QB * N * H]])
                nc.vector.tensor_scalar_max(out=flat, in0=flat, scalar1=0.0)

        # ---- w2 matvec (column form, contiguous 100-col chunks) -----
        for i in range(QB):
            b = q * QB + i
            amc = st.psm.tile([100, NC2], DT, tag="amc")
            for c in range(NC2 if "matvec" not in SKIP else 2):
                for ac in range(2):
                    lhs = st.hids[q][ac][:, i, 2 * c:2 * c + 2, :]
                    nc.tensor.matmul(amc[:, c:c + 1], lhsT=lhs,
                                     rhs=st.w2col[:, ac:ac + 1],
                                     start=(ac == 0), stop=(ac == 1))
            # evacuate + mask bias column (fp32 logits)
            amcs = st.amcp.tile([100, NC2], DT, tag="amcs")
            nc.scalar.activation(out=amcs, in_=amc, func=AF.Identity,
                                 bias=mb2[:, b:b + 1], scale=1.0)
            amT = st.pst.tile([NC2, 100], DT, tag="amT")
            nc.tensor.transpose(amT[:, :], amcs[:, :], ident[:100, :100])
            nc.scalar.copy(out=amr[:, b, :, :], in_=amT)

    # ---------------- batched softmax over h -------------------------
    for half in range(2):
        nb = BC // 2
        off = half * nb * 2 * H
        amm_v = _ap(amr, off, [list(amr.ap[0]), [2 * H, nb], [H, 2], [1, H]])
        nm_v = _ap(nm, half * nb * 2, [list(nm.ap[0]), [2, nb], [1, 2]])
        nc.vector.tensor_reduce(out=nm_v, in_=amm_v, axis=AX.X, op=ALU.max,
                                negate=True)
        am2_v = _ap(am2, off, [list(am2.ap[0]), [2 * H, nb], [H, 2], [1, H]])
        nc.vector.tensor_add(out=am2_v, in0=amm_v, in1=_bc(nm_v, 3, H))
        ex_v = _ap(ex, half * nb * 2 * 64,
                   [list(ex.ap[0]), [2 * 64, nb], [64, 2], [1, H]])
        nc.scalar.activation(out=ex_v, in_=am2_v, func=AF.Exp)
        ssum_v = _ap(rs, half * nb * 2, [list(rs.ap[0]), [2, nb], [1, 2]])
        nc.vector.tensor_reduce(out=ssum_v, in_=ex_v, axis=AX.X, op=ALU.add)
    nc.vector.reciprocal(rs, rs)

    # ---------------- attention-weighted history ----------------------
    for b in range(BC if "final" not in SKIP else 0):
        # both parities in one M=57 matmul: parity 0 -> psum rows 0:25,
        # parity 1 -> rows 32:57 (32-aligned for the evacuations)
        eT2 = st.eTp.tile([H, 57], BF, tag="eT2")
        peT = st.pse.tile([128, NC2], BF, tag="peT")
        nc.tensor.transpose(peT[:, :], ex[:, b, :, :], identB[:NC2, :NC2])
        for nr in range(2):
            nc.scalar.copy(out=eT2[:, 32 * nr:32 * nr + NC2],
                           in_=peT[64 * nr:64 * nr + H, :])
        pur = st.purp.tile([57, F], DT, tag="pur")
        nc.tensor.matmul(pur, lhsT=eT2, rhs=histf[:, b, :],
                         start=True, stop=True)
        nc.scalar.activation(out=urs[:, 0, b, :], in_=pur[0:NC2, :],
                             func=AF.Copy, scale=rs[:, b, 0:1])
        nc.vector.tensor_scalar_mul(
            out=urs[:, 1, b, :], in0=pur[32:32 + NC2, :],
            scalar1=rs[:, b, 1:2])  # DVE: tail slot
        eng = nc.sync if b % 2 == 0 else nc.scalar
        eng.dma_start(
            out=_ap(ur_out.ap(), b * N * F, [[2 * F, NC2], [F, 2], [1, F]]),
            in_=urs[:, :, b, :])


def build(debug=False, reps=1):
    import contextlib

    nc = bacc.Bacc("TRN2", target_bir_lowering=False, debug=debug)
    hist_in = nc.dram_tensor("hist_in", [BC, H, D], DT, kind="ExternalInput")
    cand_in = nc.dram_tensor("cand_in", [BC, N, D], DT, kind="ExternalInput")
    mask_in = nc.dram_tensor("mask_in", [BC, H], DT, kind="ExternalInput")
    pos_in = nc.dram_tensor("pos_in", [BC, H], I32, kind="ExternalInput")
    pos_emb = nc.dram_tensor("pos_emb", [J, P], DT, kind="ExternalInput")
    w1t = nc.dram_tensor("w1t", [2 * F, A], DT, kind="ExternalInput")
    pos_embT = nc.dram_tensor("pos_embT", [P, J], DT, kind="ExternalInput")
    b1 = nc.dram_tensor("b1", [A], DT, kind="ExternalInput")
    w2 = nc.dram_tensor("w2", [A], DT, kind="ExternalInput")
    ur_out = nc.dram_tensor("ur_out", [BC, N, F], DT, kind="ExternalOutput")
    cand_out = nc.dram_tensor("cand_out", [BC, N, F], DT, kind="ExternalOutput")

    with tile.TileContext(nc) as tc:
        with contextlib.ExitStack() as ctx:
            st = _setup(nc, tc, ctx)
            for _ in range(reps):
                _rep(nc, st, hist_in, cand_in, mask_in, pos_in, pos_emb,
                     w1t, pos_embT, b1, w2, ur_out, cand_out)
    nc.compile()
    return nc


_NC = None


def _get_nc():
    global _NC
    if _NC is None:
        _NC = build(debug=False)
    return _NC


def make_in_maps(history_repr, candidate_repr, user_history_mask,
                 user_history_position, pos_emb, W1, b1, w2):
    hist = np.ascontiguousarray(np.asarray(history_repr, np.float32))
    cand = np.ascontiguousarray(np.asarray(candidate_repr, np.float32))
    mask = np.asarray(user_history_mask).astype(np.float32)
    pos = np.asarray(user_history_position).astype(np.int32)
    pe = np.ascontiguousarray(np.asarray(pos_emb, np.float32))
    w1t = np.ascontiguousarray(np.asarray(W1, np.float32).T)
    peT = np.ascontiguousarray(pe.T)
    b1_ = np.ascontiguousarray(np.asarray(b1, np.float32))
    w2_ = np.ascontiguousarray(np.asarray(w2, np.float32))
    in_maps = []
    for c in range(NCORES):
        sl = slice(c * BC, (c + 1) * BC)
        in_maps.append({
            "hist_in": hist[sl], "cand_in": cand[sl],
            "mask_in": mask[sl], "pos_in": pos[sl],
            "pos_emb": pe, "w1t": w1t, "pos_embT": peT,
            "b1": b1_, "w2": w2_,
        })
    return in_maps


def kernel(history_repr, candidate_repr, user_history_mask,
           user_history_position, pos_emb, W1, b1, w2, b2=None, **_ignored):
    # b2 shifts every logit equally -> cancels in softmax; unused.
    nc = _get_nc()
    in_maps = make_in_maps(history_repr, candidate_repr, user_history_mask,
                           user_history_position, pos_emb, W1, b1, w2)
    res = bass_utils.run_bass_kernel_spmd(nc, in_maps, list(range(NCORES)))
    ur = np.concatenate([res.results[c]["ur_out"] for c in range(NCORES)], 0)
    cand = np.concatenate([res.results[c]["cand_out"] for c in range(NCORES)], 0)
    return ur, cand
